# revision 1
# baseline (speedup 1.0000x reference)
"""GQA attention + RoPE, tensor-parallel across 8 NeuronCores (Bass/Tile).

Model: x(1,2048,2048) -> Q=xWq (32 heads x 64), K/V=xWk/xWv (8 kv heads),
RoPE on q/k, causal softmax attention (GQA: 4 q heads per kv head), out-proj.

Sharding: head-parallel. Core i gets q heads 4i..4i+3 (Wq cols), kv head i
(Wk/Wv cols), Wo rows 256i..256i+256. Each core computes a partial (2048,2048)
output; host sums the 8 partials (the "all-reduce").

Per-core layout strategy (everything feature-on-partitions):
  xT (128,16,2048): x^T tiled by feature blocks; streamed in 512-seq chunks.
  Q^T/K^T computed directly via matmul(lhsT=W-tile, rhs=xT-chunk) -> (d, s).
  RoPE: out = q*C + swap32(q)*S with host-replicated trig tables; the
  half-swap is done with small SBUF<-PSUM DMAs (engines are partition-locked).
  Scores computed transposed: ST[t,s] = K^T-block^T @ Q^T-chunk, k=64, with
  the two heads of a pair row-packed into PE row groups 0-63/64-127.
  Softmax without max-subtraction (scores are O(5), fp32 exp is safe):
  exp on ACT fused with the 1/8 scale; causal handled by a triangular mask
  multiply on diagonal 128-blocks plus memset of fully-masked columns.
  ctx^T = V'-block^T @ expST with V' = [V | ones] so psum row 64 accumulates
  the softmax denominator for free; normalize = reciprocal + gpsimd
  partition_broadcast + multiply.
  Out-proj: psum += ctxT-tile^T @ Wo-rows, k=128 over 2 head-pair tiles.
All matmuls run as float32r (full PE rate at moving-dim>=256).
"""

import numpy as np
from contextlib import ExitStack

import concourse.bass as bass
from concourse import bacc
import concourse.tile as tile
from concourse import mybir
from concourse.bass_utils import run_bass_kernel_spmd

F32 = mybir.dt.float32
F32R = mybir.dt.float32r
AF = mybir.ActivationFunctionType

S = 2048          # sequence length
D = 2048          # model dim
HD = 64           # head dim
NCORES = 8
QH = 4            # q heads per core
QC = QH * HD      # 256 q columns per core
SC = 512          # seq chunk width
NSC = S // SC     # 4 chunks
KB = D // 128     # 16 feature blocks
SCALE = 1.0 / 8.0  # 1/sqrt(64)

_NC = None
DEBUG = False


def _build():
    nc = bacc.Bacc(None)
    xT = nc.declare_dram_parameter("xT", [128, KB, S], F32R, isOutput=False)
    wq = nc.declare_dram_parameter("wq", [128, KB, QC], F32R, isOutput=False)
    wkv = nc.declare_dram_parameter("wkv", [128, KB, 128], F32R, isOutput=False)
    wo = nc.declare_dram_parameter("wo", [128, 2, D], F32R, isOutput=False)
    ctab = nc.declare_dram_parameter("ctab", [128, S], F32, isOutput=False)
    stab = nc.declare_dram_parameter("stab", [128, S], F32, isOutput=False)
    trimask = nc.declare_dram_parameter("trimask", [128, 128], F32, isOutput=False)
    eye = nc.declare_dram_parameter("eye", [64, 64], F32, isOutput=False)
    vones = nc.declare_dram_parameter("vones", [128, KB, 2], F32R, isOutput=False)
    zpad = nc.declare_dram_parameter("zpad", [128, 384], F32R, isOutput=False)
    out = nc.declare_dram_parameter("out", [S, D], F32, isOutput=True)
    if DEBUG:
        dq0 = nc.declare_dram_parameter("dq0", [128, S], F32, isOutput=True)
        dkt = nc.declare_dram_parameter("dkt", [128, S], F32, isOutput=True)
        dv = nc.declare_dram_parameter("dv", [128, KB, HD + 2], F32, isOutput=True)
        dc0 = nc.declare_dram_parameter("dc0", [128, S], F32, isOutput=True)
        dctxr = nc.declare_dram_parameter("dctxr", [HD + 2, SC], F32, isOutput=True)
        dbc = nc.declare_dram_parameter("dbc", [64, SC], F32, isOutput=True)
        dex = nc.declare_dram_parameter("dex", [128, SC], F32, isOutput=True)

    with tile.TileContext(nc) as tc, ExitStack() as ctx:
        sb = ctx.enter_context(tc.tile_pool(name="sb", bufs=1))
        xp = ctx.enter_context(tc.tile_pool(name="xp", bufs=16))
        wk_ = ctx.enter_context(tc.tile_pool(name="wk", bufs=2))
        pp = ctx.enter_context(tc.tile_pool(name="pp", bufs=2, space="PSUM"))

        # ---- persistent constants ----
        wq_sb = sb.tile([128, KB, QC], F32R)
        nc.sync.dma_start(out=wq_sb, in_=wq[:, :, :])
        wkv_sb = sb.tile([128, KB, 128], F32R)
        nc.sync.dma_start(out=wkv_sb, in_=wkv[:, :, :])
        wo_sb = sb.tile([128, 2, D], F32R)
        nc.sync.dma_start(out=wo_sb, in_=wo[:, :, :])
        ctab_sb = sb.tile([128, S], F32)
        nc.sync.dma_start(out=ctab_sb, in_=ctab[:, :])
        stab_sb = sb.tile([128, S], F32)
        nc.sync.dma_start(out=stab_sb, in_=stab[:, :])
        tri_sb = sb.tile([128, 128], F32)
        nc.sync.dma_start(out=tri_sb, in_=trimask[:, :])
        eye_sb = sb.tile([64, 64], F32)
        nc.sync.dma_start(out=eye_sb, in_=eye[:, :])

        # ---- persistent activations ----
        qt0 = sb.tile([128, S], F32R)   # q^T heads 0,1 (roped)
        qt1 = sb.tile([128, S], F32R)   # q^T heads 2,3
        qts = [qt0, qt1]
        kt_sb = sb.tile([128, S], F32R)  # rows 0-63 k^T roped; 64-127 duplicate
        v_sb = sb.tile([128, KB, HD + 2], F32R)  # V natural (t,d) + ones col
        ct0 = sb.tile([128, S], F32R)   # normalized ctx^T heads 0,1
        ct1 = sb.tile([128, S], F32R)
        cts = [ct0, ct1]
        nc.sync.dma_start(out=v_sb[:, :, HD:HD + 2], in_=vones[:, :, :])
        zpad_sb = sb.tile([128, 384], F32R)
        nc.sync.dma_start(out=zpad_sb, in_=zpad[:, :])

        def emit_proj(c):
            cs = slice(c * SC, (c + 1) * SC)
            xc = []
            for kb in range(KB):
                t = xp.tile([128, SC], F32R, name=f"xc_{c}_{kb}", tag="xc")
                nc.sync.dma_start(out=t, in_=xT[:, kb, cs])
                xc.append(t)
            # Q projection + rope, one 128-tile (2 heads) at a time
            for u in range(2):
                pq = pp.tile([128, SC], F32, name=f"pq_{c}_{u}", tag="pq")
                for kb in range(KB):
                    nc.tensor.matmul(
                        pq,
                        lhsT=wq_sb[:, kb, u * 128:(u + 1) * 128],
                        rhs=xc[kb],
                        start=(kb == 0), stop=(kb == KB - 1),
                    )
                qraw = wk_.tile([128, SC], F32, name=f"qraw_{c}_{u}", tag="qraw")
                nc.scalar.copy(qraw, pq)
                t1 = wk_.tile([128, SC], F32, name=f"rt1_{c}_{u}", tag="rt", bufs=2)
                nc.vector.tensor_mul(t1, pq, ctab_sb[:, cs])
                qsw = wk_.tile([128, SC], F32, name=f"qsw_{c}_{u}", tag="qsw")
                for b in (0, 64):
                    nc.sync.dma_start(out=qsw[b:b + 32, :], in_=qraw[b + 32:b + 64, :])
                    nc.sync.dma_start(out=qsw[b + 32:b + 64, :], in_=qraw[b:b + 32, :])
                t2 = wk_.tile([128, SC], F32, name=f"rt2_{c}_{u}", tag="rt", bufs=2)
                nc.vector.tensor_mul(t2, qsw, stab_sb[:, cs])
                nc.vector.tensor_add(qts[u][:, cs], t1, t2)
            # KV projection
            pkv = pp.tile([128, SC], F32, name=f"pkv_{c}", tag="pq")
            for kb in range(KB):
                nc.tensor.matmul(
                    pkv,
                    lhsT=wkv_sb[:, kb, :],
                    rhs=xc[kb],
                    start=(kb == 0), stop=(kb == KB - 1),
                )
            # K rope (rows 0-63); V raw (rows 64-127) to sbuf
            kvraw = wk_.tile([128, SC], F32, name=f"kvraw_{c}", tag="qraw")
            nc.scalar.copy(kvraw, pkv)
            k1 = wk_.tile([64, SC], F32, name=f"kr1_{c}", tag="krt", bufs=2)
            nc.vector.tensor_mul(k1, pkv[0:64, :], ctab_sb[0:64, cs])
            ksw = wk_.tile([64, SC], F32, name=f"ksw_{c}", tag="ksw")
            nc.sync.dma_start(out=ksw[0:32, :], in_=kvraw[32:64, :])
            nc.sync.dma_start(out=ksw[32:64, :], in_=kvraw[0:32, :])
            k2 = wk_.tile([64, SC], F32, name=f"kr2_{c}", tag="krt", bufs=2)
            nc.vector.tensor_mul(k2, ksw, stab_sb[0:64, cs])
            nc.vector.tensor_add(kt_sb[0:64, cs], k1, k2)
            nc.sync.dma_start(out=kt_sb[64:128, cs], in_=kt_sb[0:64, cs])
            # V natural layout: shift rows 64-127 down, then PE-transpose
            # each 128-seq block
            vtr = wk_.tile([64, SC], F32, name=f"vtr_{c}", tag="vtr")
            nc.sync.dma_start(out=vtr, in_=kvraw[64:128, :])
            for r in range(4):
                j = 4 * c + r
                pt = pp.tile([128, HD], F32, name=f"pt_{c}_{r}", tag="ps")
                nc.tensor.transpose(pt, vtr[:, r * 128:(r + 1) * 128], eye_sb)
                nc.any.tensor_copy(v_sb[:, j, 0:HD], pt)

        def emit_attn(c):
            cs = slice(c * SC, (c + 1) * SC)
            for u in range(2):
                cps = []
                for idx in range(2):
                    cpt = pp.tile([HD + 2, SC], F32, name=f"cp_{c}_{u}_{idx}",
                                  tag="pc")
                    cps.append(cpt)
                njt = 4 * c + 4
                for j in range(njt):
                    diag = j >= 4 * c
                    r = j - 4 * c
                    jb = slice(j * 128, (j + 1) * 128)
                    # columns of this chunk still unmasked for t-block j
                    lo = 128 * r if diag else 0
                    nsl = slice(lo, SC)
                    csl = slice(c * SC + lo, (c + 1) * SC)
                    es = []
                    for idx in range(2):
                        sp = pp.tile([128, SC], F32, name=f"sp_{c}_{u}_{j}_{idx}",
                                     tag="ps")
                        nc.tensor.matmul(
                            sp[:, nsl],
                            lhsT=kt_sb[idx * 64:idx * 64 + 64, jb],
                            rhs=qts[u][idx * 64:idx * 64 + 64, csl],
                            start=True, stop=True,
                            tile_position=(idx * 64, 0),
                        )
                        e = wk_.tile([128, SC], F32R, name=f"e_{c}_{u}_{j}_{idx}",
                                     tag="ex", bufs=3)
                        if lo:
                            nc.sync.dma_start(out=e[:, 0:lo], in_=zpad_sb[:, 0:lo])
                        nc.scalar.activation(e[:, nsl], sp[:, nsl], AF.Exp,
                                             scale=SCALE)
                        if diag:
                            dsl = slice(lo, lo + 128)
                            nc.vector.tensor_mul(e[:, dsl], e[:, dsl], tri_sb)
                        es.append(e)
                    for idx in range(2):
                        nc.tensor.matmul(
                            cps[idx],
                            lhsT=v_sb[:, j, :],
                            rhs=es[idx],
                            start=(j == 0), stop=(j == njt - 1),
                        )
                    if DEBUG and c == 0 and u == 0 and j == 0:
                        dxt = wk_.tile([128, SC], F32, name="dxt", tag="dxt")
                        nc.vector.tensor_copy(dxt, es[0])
                        nc.sync.dma_start(out=dex[:, :], in_=dxt)
                if DEBUG and c == 0 and u == 0:
                    dcc = wk_.tile([HD + 2, SC], F32, name="dcc", tag="dcc")
                    nc.vector.tensor_copy(dcc, cps[0])
                    nc.sync.dma_start(out=dctxr[:, :], in_=dcc)
                for idx in range(2):
                    cpy = wk_.tile([HD + 2, SC], F32, name=f"cpy_{c}_{u}_{idx}",
                                   tag="cpy")
                    nc.any.tensor_copy(cpy, cps[idx])
                    den0 = wk_.tile([1, SC], F32, name=f"den_{c}_{u}_{idx}",
                                    tag="den")
                    nc.sync.dma_start(out=den0, in_=cpy[HD:HD + 1, :])
                    rec0 = wk_.tile([1, SC], F32, name=f"rec_{c}_{u}_{idx}",
                                    tag="rec")
                    nc.vector.reciprocal(rec0, den0)
                    bc = wk_.tile([64, SC], F32, name=f"bc_{c}_{u}_{idx}",
                                  tag="bc")
                    nc.gpsimd.partition_broadcast(bc, rec0)
                    if DEBUG and c == 0 and u == 0 and idx == 0:
                        nc.sync.dma_start(out=dbc[:, :], in_=bc)
                    if idx == 0:
                        nc.vector.tensor_mul(cts[u][0:64, cs], cpy[0:64, :], bc)
                    else:
                        ns = wk_.tile([64, SC], F32R, name=f"ns_{c}_{u}", tag="ns")
                        nc.vector.tensor_mul(ns, cpy[0:64, :], bc)
                        nc.sync.dma_start(out=cts[u][64:128, cs], in_=ns)

        def emit_outproj(c):
            for mi in range(4):
                m = 4 * c + mi
                mb = slice(m * 128, (m + 1) * 128)
                for n in range(4):
                    nck = slice(n * SC, (n + 1) * SC)
                    po = pp.tile([128, SC], F32, name=f"po_{c}_{mi}_{n}", tag="po")
                    for u in range(2):
                        nc.tensor.matmul(
                            po,
                            lhsT=cts[u][:, mb],
                            rhs=wo_sb[:, u, nck],
                            start=(u == 0), stop=(u == 1),
                        )
                    ob = wk_.tile([128, SC], F32, name=f"ob_{c}_{mi}_{n}",
                                  tag="ob", bufs=2)
                    nc.vector.tensor_copy(ob, po)
                    nc.sync.dma_start(out=out[mb, nck], in_=ob)

        emit_proj(0)
        for c in range(NSC):
            if c + 1 < NSC:
                emit_proj(c + 1)
            emit_attn(c)
            emit_outproj(c)
        if DEBUG:
            nc.sync.dma_start(out=dq0[:, :], in_=qt0.bitcast(F32))
            nc.sync.dma_start(out=dkt[:, :], in_=kt_sb.bitcast(F32))
            nc.sync.dma_start(out=dv[:, :, :], in_=v_sb.bitcast(F32))
            nc.sync.dma_start(out=dc0[:, :], in_=ct0.bitcast(F32))

    nc.finalize()
    return nc


def _get_nc():
    global _NC
    if _NC is None:
        _NC = _build()
    return _NC


def _prep_in_maps(x, Wq, Wk, Wv, Wo, cos, sin):
    x0 = np.ascontiguousarray(np.asarray(x, np.float32).reshape(S, D))
    xT = np.ascontiguousarray(x0.T.reshape(KB, 128, S).transpose(1, 0, 2))
    cosT = np.ascontiguousarray(np.asarray(cos, np.float32).T)  # (32, S)
    sinT = np.ascontiguousarray(np.asarray(sin, np.float32).T)
    ctab = np.tile(cosT, (4, 1))                                   # (128, S)
    stab = np.tile(np.vstack([-sinT, sinT]), (2, 1))               # (128, S)
    trimask = (np.arange(128)[:, None] <= np.arange(128)[None, :]).astype(
        np.float32)
    eye = np.eye(64, dtype=np.float32)
    vones_a = np.zeros((128, KB, 2), np.float32); vones_a[:, :, 0] = 1.0
    zpad_a = np.zeros((128, 384), np.float32)
    Wq = np.asarray(Wq, np.float32)
    Wk = np.asarray(Wk, np.float32)
    Wv = np.asarray(Wv, np.float32)
    Wo = np.asarray(Wo, np.float32)

    in_maps = []
    for i in range(NCORES):
        wq_i = np.ascontiguousarray(
            Wq[:, i * QC:(i + 1) * QC].reshape(KB, 128, QC).transpose(1, 0, 2))
        wkv_i = np.concatenate(
            [Wk[:, i * HD:(i + 1) * HD], Wv[:, i * HD:(i + 1) * HD]], axis=1)
        wkv_i = np.ascontiguousarray(
            wkv_i.reshape(KB, 128, 128).transpose(1, 0, 2))
        wo_i = np.ascontiguousarray(
            Wo[i * QC:(i + 1) * QC, :].reshape(2, 128, D).transpose(1, 0, 2))
        in_maps.append({
            "xT": xT, "wq": wq_i, "wkv": wkv_i, "wo": wo_i,
            "ctab": ctab, "stab": stab, "trimask": trimask, "eye": eye,
            "vones": vones_a, "zpad": zpad_a,
        })
    return in_maps


def run(inputs, **kw):
    nc = _get_nc()
    in_maps = _prep_in_maps(**inputs)
    return run_bass_kernel_spmd(nc, in_maps, list(range(NCORES)), **kw)


def kernel(x, Wq, Wk, Wv, Wo, cos, sin):
    res = run(dict(x=x, Wq=Wq, Wk=Wk, Wv=Wv, Wo=Wo, cos=cos, sin=sin))
    acc = np.zeros((S, D), np.float32)
    for r in res.results:
        acc += r["out"]
    return acc.reshape(1, S, D)



# revision 3
# speedup vs baseline: 1.5756x; 1.5756x over previous
"""GQA attention + RoPE, tensor-parallel across 8 NeuronCores (Bass/Tile).

Model: x(1,2048,2048) -> Q=xWq (32 heads x 64), K/V=xWk/xWv (8 kv heads),
RoPE on q/k, causal softmax attention (GQA: 4 q heads per kv head), out-proj.

Sharding: head-parallel. Core i gets q heads 4i..4i+3 (Wq cols), kv head i
(Wk/Wv cols), Wo rows 256i..256i+256. Each core computes a partial (2048,2048)
output; host sums the 8 partials (the "all-reduce").

v2 (vs 435us baseline): bf16 everywhere on PE inputs (halves DMA + DVE),
RoPE half-swap via DVE stream_shuffle (host permutes head dims so the rope
pair (x1_i,x2_i) sits 16 partitions apart inside a 32-partition quadrant -
legal for scores since q and k share the permutation), causal-trimmed ctx
matmuls, reciprocal_approx_fast for the softmax denominator, and proj/
out-proj matmuls interleaved into the attention j-loop so the PE never sees
a >3.4us gap (HAM stays at K=8/8 instead of oscillating).
"""

import numpy as np
from contextlib import ExitStack

import concourse.bass as bass
from concourse import bacc
import concourse.tile as tile
from concourse import mybir
from concourse.bass_utils import run_bass_kernel_spmd

F32 = mybir.dt.float32
BF16 = mybir.dt.bfloat16
AF = mybir.ActivationFunctionType

S = 2048          # sequence length
D = 2048          # model dim
HD = 64           # head dim
NCORES = 8
QH = 4            # q heads per core
QC = QH * HD      # 256 q columns per core
SC = 512          # seq chunk width
NSC = S // SC     # 4 chunks
KB = D // 128     # 16 feature blocks
SCALE = 1.0 / 8.0  # 1/sqrt(64)
SHUF = list(range(16, 32)) + list(range(16))  # rope pair swap, per quadrant

_NC = None


def _build():
    nc = bacc.Bacc(None)
    xT = nc.declare_dram_parameter("xT", [128, KB, S], BF16, isOutput=False)
    wq = nc.declare_dram_parameter("wq", [128, KB, QC], BF16, isOutput=False)
    wkv = nc.declare_dram_parameter("wkv", [128, KB, 128], BF16, isOutput=False)
    wo = nc.declare_dram_parameter("wo", [128, 2, D], BF16, isOutput=False)
    ctab = nc.declare_dram_parameter("ctab", [128, S], BF16, isOutput=False)
    stab = nc.declare_dram_parameter("stab", [128, S], BF16, isOutput=False)
    trimask = nc.declare_dram_parameter("trimask", [128, 128], BF16,
                                        isOutput=False)
    eye = nc.declare_dram_parameter("eye", [64, 64], BF16, isOutput=False)
    out = nc.declare_dram_parameter("out", [S, D], BF16, isOutput=True)

    with tile.TileContext(nc) as tc, ExitStack() as ctx:
        sb = ctx.enter_context(tc.tile_pool(name="sb", bufs=1))
        wk_ = ctx.enter_context(tc.tile_pool(name="wk", bufs=2))
        pp = ctx.enter_context(tc.tile_pool(name="pp", bufs=1, space="PSUM"))

        # ---- persistent constants ----
        wq_sb = sb.tile([128, KB, QC], BF16)
        nc.sync.dma_start(out=wq_sb, in_=wq[:, :, :])
        x_sb = sb.tile([128, KB, S], BF16)
        for kb in range(KB):
            nc.sync.dma_start(out=x_sb[:, kb, :], in_=xT[:, kb, :])
        wkv_sb = sb.tile([128, KB, 128], BF16)
        nc.sync.dma_start(out=wkv_sb, in_=wkv[:, :, :])
        ctab_sb = sb.tile([128, S], BF16)
        nc.sync.dma_start(out=ctab_sb, in_=ctab[:, :])
        stab_sb = sb.tile([128, S], BF16)
        nc.sync.dma_start(out=stab_sb, in_=stab[:, :])
        wo_sb = sb.tile([128, 2, D], BF16)
        nc.sync.dma_start(out=wo_sb, in_=wo[:, :, :])
        tri_sb = sb.tile([128, 128], BF16)
        nc.sync.dma_start(out=tri_sb, in_=trimask[:, :])
        eye_sb = sb.tile([64, 64], BF16)
        nc.sync.dma_start(out=eye_sb, in_=eye[:, :])

        # ---- persistent activations ----
        qt0 = sb.tile([128, S], BF16)   # q^T heads 0,1 (roped)
        qt1 = sb.tile([128, S], BF16)   # q^T heads 2,3
        qts = [qt0, qt1]
        kt_sb = sb.tile([128, S], BF16)  # rows 0-63 k^T roped; 64-127 dup
        v_sb = sb.tile([128, KB, HD + 2], BF16)  # V natural + [ones, 0] cols
        ct0 = sb.tile([128, S], BF16)   # normalized ctx^T heads 0,1
        ct1 = sb.tile([128, S], BF16)
        cts = [ct0, ct1]
        nc.vector.memset(v_sb[:, :, HD:HD + 1], 1.0)
        nc.vector.memset(v_sb[:, :, HD + 1:HD + 2], 0.0)

        def emit_proj_q(c, u):
            """Q projection + rope for u-tile (2 heads) of chunk c."""
            cs = slice(c * SC, (c + 1) * SC)
            pq = pp.tile([128, SC], F32, name=f"pq_{c}_{u}", tag="ppq", bufs=1)
            for kb in range(KB):
                nc.tensor.matmul(
                    pq,
                    lhsT=wq_sb[:, kb, u * 128:(u + 1) * 128],
                    rhs=x_sb[:, kb, cs],
                    start=(kb == 0), stop=(kb == KB - 1),
                )
            qraw = wk_.tile([128, SC], BF16, name=f"qraw_{c}_{u}", tag="qraw",
                            bufs=2)
            nc.vector.tensor_copy(qraw, pq)
            qsw = wk_.tile([128, SC], BF16, name=f"qsw_{c}_{u}", tag="qsw",
                           bufs=2)
            nc.vector.stream_shuffle(qsw, qraw, SHUF)
            t1 = wk_.tile([128, SC], BF16, name=f"rt1_{c}_{u}", tag="rt1",
                          bufs=2)
            nc.vector.tensor_mul(t1, qraw, ctab_sb[:, cs])
            t2 = wk_.tile([128, SC], BF16, name=f"rt2_{c}_{u}", tag="rt2",
                          bufs=2)
            nc.vector.tensor_mul(t2, qsw, stab_sb[:, cs])
            nc.vector.tensor_add(qts[u][:, cs], t1, t2)

        def emit_proj_kv(c):
            """K/V projection for chunk c: rope K (+dup), V to natural."""
            cs = slice(c * SC, (c + 1) * SC)
            pkv = pp.tile([128, SC], F32, name=f"pkv_{c}", tag="ppq", bufs=1)
            for kb in range(KB):
                nc.tensor.matmul(
                    pkv,
                    lhsT=wkv_sb[:, kb, :],
                    rhs=x_sb[:, kb, cs],
                    start=(kb == 0), stop=(kb == KB - 1),
                )
            kvraw = wk_.tile([128, SC], BF16, name=f"kvraw_{c}", tag="qraw",
                             bufs=2)
            nc.vector.tensor_copy(kvraw, pkv)
            ksw = wk_.tile([64, SC], BF16, name=f"ksw_{c}", tag="ksw", bufs=2)
            nc.vector.stream_shuffle(ksw, kvraw[0:64, :], SHUF)
            k1 = wk_.tile([64, SC], BF16, name=f"kr1_{c}", tag="kr1", bufs=2)
            nc.vector.tensor_mul(k1, kvraw[0:64, :], ctab_sb[0:64, cs])
            k2 = wk_.tile([64, SC], BF16, name=f"kr2_{c}", tag="kr2", bufs=2)
            nc.vector.tensor_mul(k2, ksw, stab_sb[0:64, cs])
            nc.vector.tensor_add(kt_sb[0:64, cs], k1, k2)
            nc.sync.dma_start(out=kt_sb[64:128, cs], in_=kt_sb[0:64, cs])
            # V natural layout: move rows 64-127 down, PE-transpose per block
            vtr = wk_.tile([64, SC], BF16, name=f"vtr_{c}", tag="vtr", bufs=2)
            nc.sync.dma_start(out=vtr, in_=kvraw[64:128, :])
            for r in range(4):
                j = 4 * c + r
                pt = pp.tile([128, HD], BF16, name=f"pt_{c}_{r}", tag="sp",
                             bufs=2)
                nc.tensor.transpose(pt, vtr[:, r * 128:(r + 1) * 128], eye_sb)
                nc.vector.tensor_copy(v_sb[:, j, 0:HD], pt)

        def emit_outproj_m(c, mi):
            """One 128-query row block of the out projection for chunk c."""
            m = 4 * c + mi
            mb = slice(m * 128, (m + 1) * 128)
            ob = wk_.tile([128, D], BF16, name=f"ob_{c}_{mi}", tag="ob",
                          bufs=2)
            for n in range(4):
                nck = slice(n * SC, (n + 1) * SC)
                po = pp.tile([128, SC], F32, name=f"po_{c}_{mi}_{n}", tag="po",
                             bufs=1)
                for u in range(2):
                    nc.tensor.matmul(
                        po,
                        lhsT=cts[u][:, mb],
                        rhs=wo_sb[:, u, nck],
                        start=(u == 0), stop=(u == 1),
                    )
                nc.vector.tensor_copy(ob[:, nck], po)
            nc.sync.dma_start(out=out[mb, :], in_=ob)

        def emit_attn(c, fillers):
            """Attention for chunk c; pops filler emitters to keep PE busy."""
            njt = 4 * c + 4
            heads = [(u, idx) for u in (0, 1) for idx in (0, 1)]
            cps = {}
            for u, idx in heads:
                cps[(u, idx)] = pp.tile([HD + 2, SC], F32,
                                        name=f"cp_{c}_{u}_{idx}",
                                        tag=f"ctx{2 * u + idx}", bufs=1)
            es_for = {}

            def emit_scores(j):
                diag = j >= 4 * c
                r = j - 4 * c
                jb = slice(j * 128, (j + 1) * 128)
                lo = 128 * r if diag else 0
                nsl = slice(lo, SC)
                csl = slice(c * SC + lo, (c + 1) * SC)
                for u in (0, 1):
                    sps = []
                    for idx in (0, 1):
                        sp = pp.tile([128, SC], F32,
                                     name=f"sp_{c}_{u}_{j}_{idx}",
                                     tag="sp", bufs=2)
                        nc.tensor.matmul(
                            sp[:, nsl],
                            lhsT=kt_sb[idx * 64:idx * 64 + 64, jb],
                            rhs=qts[u][idx * 64:idx * 64 + 64, csl],
                            start=True, stop=True,
                            tile_position=(idx * 64, 0),
                        )
                        sps.append(sp)
                    for idx in (0, 1):
                        e = wk_.tile([128, SC], BF16,
                                     name=f"e_{c}_{u}_{j}_{idx}",
                                     tag="es", bufs=8)
                        nc.scalar.activation(e[:, nsl], sps[idx][:, nsl],
                                             AF.Exp, scale=SCALE)
                        if diag:
                            dsl = slice(lo, lo + 128)
                            nc.vector.tensor_mul(e[:, dsl], e[:, dsl], tri_sb)
                        es_for[(u, idx, j)] = (e, nsl)

            def emit_ctx(j):
                for u, idx in heads:
                    e, nsl = es_for.pop((u, idx, j))
                    nc.tensor.matmul(
                        cps[(u, idx)][:, nsl],
                        lhsT=v_sb[:, j, :],
                        rhs=e[:, nsl],
                        start=(j == 0), stop=(j == njt - 1),
                    )

            emit_scores(0)
            for j in range(njt):
                if j + 1 < njt:
                    emit_scores(j + 1)
                emit_ctx(j)
                if fillers and j % 2 == 1:
                    fillers.pop(0)()
            # normalize: cts = ctx / den via recip-broadcast-multiply
            cs = slice(c * SC, (c + 1) * SC)
            for u, idx in heads:
                cp = cps[(u, idx)]
                scr = wk_.tile([HD + 1, SC], F32,
                               name=f"scr_{c}_{u}_{idx}", tag="scr", bufs=4)
                nc.scalar.copy(scr[HD:HD + 1, :], cp[HD:HD + 1, :])
                den0 = wk_.tile([1, SC], F32, name=f"den_{c}_{u}_{idx}",
                                tag="den", bufs=4)
                nc.sync.dma_start(out=den0, in_=scr[HD:HD + 1, :])
                rec0 = wk_.tile([1, SC], F32, name=f"rec_{c}_{u}_{idx}",
                                tag="rec", bufs=4)
                nc.vector.reciprocal_approx_fast(out=rec0, in_=den0)
                bcf = wk_.tile([64, SC], F32, name=f"bcf_{c}_{u}_{idx}",
                               tag="bcf", bufs=4)
                nc.gpsimd.partition_broadcast(bcf, rec0[0:1, :])
                rsl = slice(idx * 64, idx * 64 + 64)
                nc.vector.scalar_tensor_tensor(
                    cts[u][rsl, cs], cp[0:HD, :], 1.0, bcf,
                    mybir.AluOpType.mult, mybir.AluOpType.mult,
                )
            while fillers:
                fillers.pop(0)()

        # ---- schedule ----
        emit_proj_q(0, 0)
        emit_proj_q(0, 1)
        emit_proj_kv(0)
        for c in range(NSC):
            fillers = []
            if c + 1 < NSC:
                fillers.append(lambda cc=c + 1: emit_proj_q(cc, 0))
            if c > 0:
                fillers.append(lambda cc=c - 1: emit_outproj_m(cc, 0))
            if c + 1 < NSC:
                fillers.append(lambda cc=c + 1: emit_proj_q(cc, 1))
            if c > 0:
                fillers.append(lambda cc=c - 1: emit_outproj_m(cc, 1))
            if c + 1 < NSC:
                fillers.append(lambda cc=c + 1: emit_proj_kv(cc))
            if c > 0:
                fillers.append(lambda cc=c - 1: emit_outproj_m(cc, 2))
                fillers.append(lambda cc=c - 1: emit_outproj_m(cc, 3))
            emit_attn(c, fillers)
        for mi in range(4):
            emit_outproj_m(NSC - 1, mi)

    nc.finalize()
    return nc


def _get_nc():
    global _NC
    if _NC is None:
        _NC = _build()
    return _NC


def _rope_perm():
    """Head-local (64) permutation: pair (x1_i, x2_i) -> 16 apart in a
    32-partition quadrant. newpos[old] for old in 0..63."""
    newpos = np.empty(64, dtype=np.int64)
    for i in range(32):
        newpos[i] = (i // 16) * 32 + (i % 16)           # x1_i
        newpos[32 + i] = (i // 16) * 32 + 16 + (i % 16)  # x2_i
    return newpos


def _prep_in_maps(x, Wq, Wk, Wv, Wo, cos, sin):
    import ml_dtypes
    bf = ml_dtypes.bfloat16
    x0 = np.asarray(x, np.float32).reshape(S, D)
    xT = np.ascontiguousarray(
        x0.T.reshape(KB, 128, S).transpose(1, 0, 2)).astype(bf)

    newpos = _rope_perm()
    # permutation as gather: perm_src[new] = old
    perm_src = np.empty(64, dtype=np.int64)
    perm_src[newpos] = np.arange(64)

    # rope tables in the permuted layout (pattern has period 64)
    cosT = np.asarray(cos, np.float32).T  # (32, S)
    sinT = np.asarray(sin, np.float32).T
    ctab64 = np.empty((64, S), np.float32)
    stab64 = np.empty((64, S), np.float32)
    for p in range(64):
        quad, off = p // 32, p % 32
        i = quad * 16 + (off % 16)
        is_x2 = off >= 16
        ctab64[p] = cosT[i]
        stab64[p] = sinT[i] if is_x2 else -sinT[i]
    ctab = np.tile(ctab64, (2, 1)).astype(bf)
    stab = np.tile(stab64, (2, 1)).astype(bf)

    trimask = (np.arange(128)[:, None] <= np.arange(128)[None, :]).astype(bf)
    eye = np.eye(64, dtype=np.float32).astype(bf)

    Wq = np.asarray(Wq, np.float32)
    Wk = np.asarray(Wk, np.float32)
    Wv = np.asarray(Wv, np.float32)
    Wo = np.asarray(Wo, np.float32)
    # apply rope perm within each head's 64 columns
    Wq_p = Wq.reshape(D, 32, 64)[:, :, perm_src].reshape(D, D)
    Wk_p = Wk.reshape(D, 8, 64)[:, :, perm_src].reshape(D, 8 * 64)

    in_maps = []
    for i in range(NCORES):
        wq_i = np.ascontiguousarray(
            Wq_p[:, i * QC:(i + 1) * QC].reshape(KB, 128, QC)
            .transpose(1, 0, 2)).astype(bf)
        wkv_i = np.concatenate(
            [Wk_p[:, i * HD:(i + 1) * HD], Wv[:, i * HD:(i + 1) * HD]],
            axis=1)
        wkv_i = np.ascontiguousarray(
            wkv_i.reshape(KB, 128, 128).transpose(1, 0, 2)).astype(bf)
        wo_i = np.ascontiguousarray(
            Wo[i * QC:(i + 1) * QC, :].reshape(2, 128, D)
            .transpose(1, 0, 2)).astype(bf)
        in_maps.append({
            "xT": xT, "wq": wq_i, "wkv": wkv_i, "wo": wo_i,
            "ctab": ctab, "stab": stab, "trimask": trimask, "eye": eye,
        })
    return in_maps


def run(inputs, **kw):
    nc = _get_nc()
    in_maps = _prep_in_maps(**inputs)
    return run_bass_kernel_spmd(nc, in_maps, list(range(NCORES)), **kw)


def kernel(x, Wq, Wk, Wv, Wo, cos, sin):
    res = run(dict(x=x, Wq=Wq, Wk=Wk, Wv=Wv, Wo=Wo, cos=cos, sin=sin))
    acc = np.zeros((S, D), np.float32)
    for r in res.results:
        acc += r["out"].astype(np.float32)
    return acc.reshape(1, S, D)


# revision 8
# speedup vs baseline: 1.6295x; 1.0342x over previous
"""GQA attention + RoPE, tensor-parallel across 8 NeuronCores (Bass/Tile).

Model: x(1,2048,2048) -> Q=xWq (32 heads x 64), K/V=xWk/xWv (8 kv heads),
RoPE on q/k, causal softmax attention (GQA: 4 q heads per kv head), out-proj.

Sharding: head-parallel. Core i gets q heads 4i..4i+3 (Wq cols), kv head i
(Wk/Wv cols), Wo rows 256i..256i+256. Each core computes a partial (2048,2048)
output; host sums the 8 partials (the "all-reduce").

v2 (vs 435us baseline): bf16 everywhere on PE inputs (halves DMA + DVE),
RoPE half-swap via DVE stream_shuffle (host permutes head dims so the rope
pair (x1_i,x2_i) sits 16 partitions apart inside a 32-partition quadrant -
legal for scores since q and k share the permutation), causal-trimmed ctx
matmuls, reciprocal_approx_fast for the softmax denominator, and proj/
out-proj matmuls interleaved into the attention j-loop so the PE never sees
a >3.4us gap (HAM stays at K=8/8 instead of oscillating).
"""

import numpy as np
from contextlib import ExitStack

import concourse.bass as bass
from concourse import bacc
import concourse.tile as tile
from concourse import mybir
from concourse.bass_utils import run_bass_kernel_spmd

F32 = mybir.dt.float32
BF16 = mybir.dt.bfloat16
AF = mybir.ActivationFunctionType

S = 2048          # sequence length
D = 2048          # model dim
HD = 64           # head dim
NCORES = 8
QH = 4            # q heads per core
QC = QH * HD      # 256 q columns per core
SC = 512          # seq chunk width
NSC = S // SC     # 4 chunks
KB = D // 128     # 16 feature blocks
SCALE = 1.0 / 8.0  # 1/sqrt(64)
SHUF = list(range(16, 32)) + list(range(16))  # rope pair swap, per quadrant

_NC = None


def _build():
    nc = bacc.Bacc(None)
    xT = nc.declare_dram_parameter("xT", [128, KB, S], BF16, isOutput=False)
    wq = nc.declare_dram_parameter("wq", [128, KB, QC], BF16, isOutput=False)
    wkv = nc.declare_dram_parameter("wkv", [128, KB, 128], BF16, isOutput=False)
    wo = nc.declare_dram_parameter("wo", [128, 2, D], BF16, isOutput=False)
    ctab = nc.declare_dram_parameter("ctab", [128, S], BF16, isOutput=False)
    stab = nc.declare_dram_parameter("stab", [128, S], BF16, isOutput=False)
    trimask = nc.declare_dram_parameter("trimask", [128, 128], BF16,
                                        isOutput=False)
    eye = nc.declare_dram_parameter("eye", [64, 64], BF16, isOutput=False)
    out = nc.declare_dram_parameter("out", [S, D], BF16, isOutput=True)

    with tile.TileContext(nc) as tc, ExitStack() as ctx:
        sb = ctx.enter_context(tc.tile_pool(name="sb", bufs=1))
        wk_ = ctx.enter_context(tc.tile_pool(name="wk", bufs=2))
        pp = ctx.enter_context(tc.tile_pool(name="pp", bufs=1, space="PSUM"))

        # ---- persistent constants ----
        eye_sb = sb.tile([64, 64], BF16)
        nc.sync.dma_start(out=eye_sb, in_=eye[:, :])
        wq_sb = sb.tile([128, KB, QC], BF16)
        x_sb = sb.tile([128, KB, S], BF16)
        for kb in range(KB):
            nc.sync.dma_start(out=x_sb[:, kb, :], in_=xT[:, kb, :])
            nc.sync.dma_start(out=wq_sb[:, kb, :], in_=wq[:, kb, :])
        wkv_sb = sb.tile([128, KB, 128], BF16)
        nc.sync.dma_start(out=wkv_sb, in_=wkv[:, :, :])
        ctab_sb = sb.tile([128, S], BF16)
        nc.sync.dma_start(out=ctab_sb, in_=ctab[:, :])
        stab_sb = sb.tile([128, S], BF16)
        nc.sync.dma_start(out=stab_sb, in_=stab[:, :])
        wo_sb = sb.tile([128, 2, D], BF16)
        nc.sync.dma_start(out=wo_sb, in_=wo[:, :, :])
        tri_sb = sb.tile([128, 128], BF16)
        nc.sync.dma_start(out=tri_sb, in_=trimask[:, :])

        # PE warmup spin: keep the PE busy from t~1us so the HAM clock gate
        # opens (K=8/8) before the projection matmuls start, and bridge the
        # DMA-gated prologue.
        warm = pp.tile([64, 64], F32, name="warm", tag="po", bufs=1)
        for _ in range(48):
            nc.tensor.matmul(warm, lhsT=eye_sb, rhs=eye_sb,
                             start=True, stop=True)

        # ---- persistent activations ----
        qt0 = sb.tile([128, S], BF16)   # q^T heads 0,1 (roped)
        qt1 = sb.tile([128, S], BF16)   # q^T heads 2,3
        qts = [qt0, qt1]
        kt_sb = sb.tile([128, S], BF16)  # rows 0-63 k^T roped; 64-127 dup
        v_sb = sb.tile([128, KB, HD + 2], BF16)  # V natural + [ones, 0] cols
        ct0 = sb.tile([128, S], BF16)   # normalized ctx^T heads 0,1
        ct1 = sb.tile([128, S], BF16)
        cts = [ct0, ct1]
        nc.vector.memset(v_sb[:, :, HD:HD + 1], 1.0)
        nc.vector.memset(v_sb[:, :, HD + 1:HD + 2], 0.0)

        def emit_proj_q(c, u):
            """Q projection + rope for u-tile (2 heads) of chunk c."""
            cs = slice(c * SC, (c + 1) * SC)
            pq = pp.tile([128, SC], F32, name=f"pq_{c}_{u}", tag="ppq", bufs=1)
            for kb in range(KB):
                nc.tensor.matmul(
                    pq,
                    lhsT=wq_sb[:, kb, u * 128:(u + 1) * 128],
                    rhs=x_sb[:, kb, cs],
                    start=(kb == 0), stop=(kb == KB - 1),
                )
            qraw = wk_.tile([128, SC], BF16, name=f"qraw_{c}_{u}", tag="qraw",
                            bufs=2)
            nc.vector.tensor_copy(qraw, pq)
            qsw = wk_.tile([128, SC], BF16, name=f"qsw_{c}_{u}", tag="qsw",
                           bufs=2)
            nc.vector.stream_shuffle(qsw, qraw, SHUF)
            t1 = wk_.tile([128, SC], BF16, name=f"rt1_{c}_{u}", tag="rt1",
                          bufs=2)
            nc.vector.tensor_mul(t1, qraw, ctab_sb[:, cs])
            t2 = wk_.tile([128, SC], BF16, name=f"rt2_{c}_{u}", tag="rt2",
                          bufs=2)
            nc.vector.tensor_mul(t2, qsw, stab_sb[:, cs])
            nc.vector.tensor_add(qts[u][:, cs], t1, t2)

        def emit_proj_kv(c):
            """K/V projection for chunk c: rope K (+dup), V to natural."""
            cs = slice(c * SC, (c + 1) * SC)
            pkv = pp.tile([128, SC], F32, name=f"pkv_{c}", tag="ppq", bufs=1)
            for kb in range(KB):
                nc.tensor.matmul(
                    pkv,
                    lhsT=wkv_sb[:, kb, :],
                    rhs=x_sb[:, kb, cs],
                    start=(kb == 0), stop=(kb == KB - 1),
                )
            kvraw = wk_.tile([128, SC], BF16, name=f"kvraw_{c}", tag="qraw",
                             bufs=2)
            nc.vector.tensor_copy(kvraw, pkv)
            ksw = wk_.tile([64, SC], BF16, name=f"ksw_{c}", tag="ksw", bufs=2)
            nc.vector.stream_shuffle(ksw, kvraw[0:64, :], SHUF)
            k1 = wk_.tile([64, SC], BF16, name=f"kr1_{c}", tag="kr1", bufs=2)
            nc.vector.tensor_mul(k1, kvraw[0:64, :], ctab_sb[0:64, cs])
            k2 = wk_.tile([64, SC], BF16, name=f"kr2_{c}", tag="kr2", bufs=2)
            nc.vector.tensor_mul(k2, ksw, stab_sb[0:64, cs])
            nc.vector.tensor_add(kt_sb[0:64, cs], k1, k2)
            nc.sync.dma_start(out=kt_sb[64:128, cs], in_=kt_sb[0:64, cs])
            # V natural layout: move rows 64-127 down, PE-transpose per block
            vtr = wk_.tile([64, SC], BF16, name=f"vtr_{c}", tag="vtr", bufs=2)
            nc.sync.dma_start(out=vtr, in_=kvraw[64:128, :])
            for r in range(4):
                j = 4 * c + r
                pt = pp.tile([128, HD], BF16, name=f"pt_{c}_{r}", tag="sp",
                             bufs=2)
                nc.tensor.transpose(pt, vtr[:, r * 128:(r + 1) * 128], eye_sb)
                nc.vector.tensor_copy(v_sb[:, j, 0:HD], pt)

        def emit_outproj_half(c, mi, half):
            """Half (2 n-tiles) of one 128-query row block of the out proj."""
            m = 4 * c + mi
            mb = slice(m * 128, (m + 1) * 128)
            ob = wk_.tile([128, 2 * SC], BF16, name=f"ob_{c}_{mi}_{half}",
                          tag="ob", bufs=2)
            for ni in range(2):
                n = 2 * half + ni
                nck = slice(n * SC, (n + 1) * SC)
                po = pp.tile([128, SC], F32, name=f"po_{c}_{mi}_{n}", tag="po",
                             bufs=1)
                for u in range(2):
                    nc.tensor.matmul(
                        po,
                        lhsT=cts[u][:, mb],
                        rhs=wo_sb[:, u, nck],
                        start=(u == 0), stop=(u == 1),
                    )
                nc.vector.tensor_copy(ob[:, ni * SC:(ni + 1) * SC], po)
            nc.sync.dma_start(out=out[mb, half * 2 * SC:(half + 1) * 2 * SC],
                              in_=ob)

        def emit_attn(c, fillers):
            """Attention for chunk c; pops filler emitters to keep PE busy."""
            njt = 4 * c + 4
            heads = [(u, idx) for u in (0, 1) for idx in (0, 1)]
            cps = {}
            for u, idx in heads:
                cps[(u, idx)] = pp.tile([HD + 2, SC], F32,
                                        name=f"cp_{c}_{u}_{idx}",
                                        tag=f"ctx{2 * u + idx}", bufs=1)
            es_for = {}

            def emit_scores(j):
                diag = j >= 4 * c
                r = j - 4 * c
                jb = slice(j * 128, (j + 1) * 128)
                lo = 128 * r if diag else 0
                nsl = slice(lo, SC)
                csl = slice(c * SC + lo, (c + 1) * SC)
                for u in (0, 1):
                    sps = []
                    for idx in (0, 1):
                        sp = pp.tile([128, SC], F32,
                                     name=f"sp_{c}_{u}_{j}_{idx}",
                                     tag="sp", bufs=2)
                        nc.tensor.matmul(
                            sp[:, nsl],
                            lhsT=kt_sb[idx * 64:idx * 64 + 64, jb],
                            rhs=qts[u][idx * 64:idx * 64 + 64, csl],
                            start=True, stop=True,
                            tile_position=(idx * 64, 0),
                        )
                        sps.append(sp)
                    for idx in (0, 1):
                        e = wk_.tile([128, SC], BF16,
                                     name=f"e_{c}_{u}_{j}_{idx}",
                                     tag="es", bufs=8)
                        nc.scalar.activation(e[:, nsl], sps[idx][:, nsl],
                                             AF.Exp, scale=SCALE)
                        if diag:
                            dsl = slice(lo, lo + 128)
                            nc.vector.tensor_mul(e[:, dsl], e[:, dsl], tri_sb)
                        es_for[(u, idx, j)] = (e, nsl)

            def emit_ctx(j):
                for u, idx in heads:
                    e, nsl = es_for.pop((u, idx, j))
                    nc.tensor.matmul(
                        cps[(u, idx)][:, nsl],
                        lhsT=v_sb[:, j, :],
                        rhs=e[:, nsl],
                        start=(j == 0), stop=(j == njt - 1),
                    )

            emit_scores(0)
            for j in range(njt):
                if j + 1 < njt:
                    emit_scores(j + 1)
                emit_ctx(j)
                if fillers:
                    fillers.pop(0)()
            # normalize: cts = ctx / den via recip-broadcast-multiply
            cs = slice(c * SC, (c + 1) * SC)
            for u, idx in heads:
                cp = cps[(u, idx)]
                scr = wk_.tile([HD + 1, SC], F32,
                               name=f"scr_{c}_{u}_{idx}", tag="scr", bufs=4)
                nc.scalar.copy(scr[HD:HD + 1, :], cp[HD:HD + 1, :])
                den0 = wk_.tile([1, SC], F32, name=f"den_{c}_{u}_{idx}",
                                tag="den", bufs=4)
                nc.sync.dma_start(out=den0, in_=scr[HD:HD + 1, :])
                rec0 = wk_.tile([1, SC], F32, name=f"rec_{c}_{u}_{idx}",
                                tag="rec", bufs=4)
                nc.vector.reciprocal_approx_fast(out=rec0, in_=den0)
                bcf = wk_.tile([64, SC], F32, name=f"bcf_{c}_{u}_{idx}",
                               tag="bcf", bufs=4)
                nc.gpsimd.partition_broadcast(bcf, rec0[0:1, :])
                rsl = slice(idx * 64, idx * 64 + 64)
                nc.vector.scalar_tensor_tensor(
                    cts[u][rsl, cs], cp[0:HD, :], 1.0, bcf,
                    mybir.AluOpType.mult, mybir.AluOpType.mult,
                )
            while fillers:
                fillers.pop(0)()

        # ---- schedule ----
        # proj(0)+proj(1) upfront: dense PE work that warms the HAM while x
        # streams in; proj(c+2) + outproj(c-1) interleave into attn(c)'s
        # j-loop, weighted toward the later (longer, exp-bound) chunks.
        for cc in (0, 1):
            emit_proj_q(cc, 0)
            emit_proj_q(cc, 1)
            emit_proj_kv(cc)
        for c in range(NSC):
            fillers = []
            if c + 2 < NSC:
                fillers.append(lambda cc=c + 2: emit_proj_q(cc, 0))
                fillers.append(lambda cc=c + 2: emit_proj_q(cc, 1))
                fillers.append(lambda cc=c + 2: emit_proj_kv(cc))
            if c > 0:
                for mi in range(4):
                    for half in (0, 1):
                        fillers.append(
                            lambda cc=c - 1, m=mi, h=half:
                            emit_outproj_half(cc, m, h))
            # alternate proj/outproj units so neither starves
            if len(fillers) > 8:
                proj_f, op_f = fillers[:3], fillers[3:]
                mixed = []
                while proj_f or op_f:
                    if op_f:
                        mixed.append(op_f.pop(0))
                    if op_f:
                        mixed.append(op_f.pop(0))
                    if proj_f:
                        mixed.append(proj_f.pop(0))
                fillers = mixed
            emit_attn(c, fillers)
        for mi in range(4):
            for half in (0, 1):
                emit_outproj_half(NSC - 1, mi, half)

    nc.finalize()
    return nc


def _get_nc():
    global _NC
    if _NC is None:
        _NC = _build()
    return _NC


def _rope_perm():
    """Head-local (64) permutation: pair (x1_i, x2_i) -> 16 apart in a
    32-partition quadrant. newpos[old] for old in 0..63."""
    newpos = np.empty(64, dtype=np.int64)
    for i in range(32):
        newpos[i] = (i // 16) * 32 + (i % 16)           # x1_i
        newpos[32 + i] = (i // 16) * 32 + 16 + (i % 16)  # x2_i
    return newpos


def _prep_in_maps(x, Wq, Wk, Wv, Wo, cos, sin):
    import ml_dtypes
    bf = ml_dtypes.bfloat16
    x0 = np.asarray(x, np.float32).reshape(S, D)
    xT = np.ascontiguousarray(
        x0.T.reshape(KB, 128, S).transpose(1, 0, 2)).astype(bf)

    newpos = _rope_perm()
    # permutation as gather: perm_src[new] = old
    perm_src = np.empty(64, dtype=np.int64)
    perm_src[newpos] = np.arange(64)

    # rope tables in the permuted layout (pattern has period 64)
    cosT = np.asarray(cos, np.float32).T  # (32, S)
    sinT = np.asarray(sin, np.float32).T
    ctab64 = np.empty((64, S), np.float32)
    stab64 = np.empty((64, S), np.float32)
    for p in range(64):
        quad, off = p // 32, p % 32
        i = quad * 16 + (off % 16)
        is_x2 = off >= 16
        ctab64[p] = cosT[i]
        stab64[p] = sinT[i] if is_x2 else -sinT[i]
    ctab = np.tile(ctab64, (2, 1)).astype(bf)
    stab = np.tile(stab64, (2, 1)).astype(bf)

    trimask = (np.arange(128)[:, None] <= np.arange(128)[None, :]).astype(bf)
    eye = np.eye(64, dtype=np.float32).astype(bf)

    Wq = np.asarray(Wq, np.float32)
    Wk = np.asarray(Wk, np.float32)
    Wv = np.asarray(Wv, np.float32)
    Wo = np.asarray(Wo, np.float32)
    # apply rope perm within each head's 64 columns
    Wq_p = Wq.reshape(D, 32, 64)[:, :, perm_src].reshape(D, D)
    Wk_p = Wk.reshape(D, 8, 64)[:, :, perm_src].reshape(D, 8 * 64)

    in_maps = []
    for i in range(NCORES):
        wq_i = np.ascontiguousarray(
            Wq_p[:, i * QC:(i + 1) * QC].reshape(KB, 128, QC)
            .transpose(1, 0, 2)).astype(bf)
        wkv_i = np.concatenate(
            [Wk_p[:, i * HD:(i + 1) * HD], Wv[:, i * HD:(i + 1) * HD]],
            axis=1)
        wkv_i = np.ascontiguousarray(
            wkv_i.reshape(KB, 128, 128).transpose(1, 0, 2)).astype(bf)
        wo_i = np.ascontiguousarray(
            Wo[i * QC:(i + 1) * QC, :].reshape(2, 128, D)
            .transpose(1, 0, 2)).astype(bf)
        in_maps.append({
            "xT": xT, "wq": wq_i, "wkv": wkv_i, "wo": wo_i,
            "ctab": ctab, "stab": stab, "trimask": trimask, "eye": eye,
        })
    return in_maps


def run(inputs, **kw):
    nc = _get_nc()
    in_maps = _prep_in_maps(**inputs)
    return run_bass_kernel_spmd(nc, in_maps, list(range(NCORES)), **kw)


def kernel(x, Wq, Wk, Wv, Wo, cos, sin):
    res = run(dict(x=x, Wq=Wq, Wk=Wk, Wv=Wv, Wo=Wo, cos=cos, sin=sin))
    acc = np.zeros((S, D), np.float32)
    for r in res.results:
        acc += r["out"].astype(np.float32)
    return acc.reshape(1, S, D)


# revision 13
# speedup vs baseline: 1.6899x; 1.0371x over previous
"""GQA attention + RoPE, tensor-parallel across 8 NeuronCores (Bass/Tile).

Model: x(1,2048,2048) -> Q=xWq (32 heads x 64), K/V=xWk/xWv (8 kv heads),
RoPE on q/k, causal softmax attention (GQA: 4 q heads per kv head), out-proj.

Sharding: head-parallel. Core i gets q heads 4i..4i+3 (Wq cols), kv head i
(Wk/Wv cols), Wo rows 256i..256i+256. Each core computes a partial (2048,2048)
output; host sums the 8 partials (the "all-reduce").

v2 (vs 435us baseline): bf16 everywhere on PE inputs (halves DMA + DVE),
RoPE half-swap via DVE stream_shuffle (host permutes head dims so the rope
pair (x1_i,x2_i) sits 16 partitions apart inside a 32-partition quadrant -
legal for scores since q and k share the permutation), causal-trimmed ctx
matmuls, reciprocal_approx_fast for the softmax denominator, and proj/
out-proj matmuls interleaved into the attention j-loop so the PE never sees
a >3.4us gap (HAM stays at K=8/8 instead of oscillating).
"""

import numpy as np
from contextlib import ExitStack

import concourse.bass as bass
from concourse import bacc
import concourse.tile as tile
from concourse import mybir
from concourse.bass_utils import run_bass_kernel_spmd

F32 = mybir.dt.float32
BF16 = mybir.dt.bfloat16
AF = mybir.ActivationFunctionType

S = 2048          # sequence length
D = 2048          # model dim
HD = 64           # head dim
NCORES = 8
QH = 4            # q heads per core
QC = QH * HD      # 256 q columns per core
SC = 512          # seq chunk width
NSC = S // SC     # 4 chunks
KB = D // 128     # 16 feature blocks
SCALE = 1.0 / 8.0  # 1/sqrt(64)
SHUF = list(range(16, 32)) + list(range(16))  # rope pair swap, per quadrant

_NC = None


def _build():
    nc = bacc.Bacc(None)
    xT = nc.declare_dram_parameter("xT", [128, KB, S], BF16, isOutput=False)
    wq = nc.declare_dram_parameter("wq", [128, KB, QC], BF16, isOutput=False)
    wkv = nc.declare_dram_parameter("wkv", [128, KB, 128], BF16, isOutput=False)
    wo = nc.declare_dram_parameter("wo", [128, 2, D], BF16, isOutput=False)
    ctab = nc.declare_dram_parameter("ctab", [128, S], BF16, isOutput=False)
    stab = nc.declare_dram_parameter("stab", [128, S], BF16, isOutput=False)
    trimask = nc.declare_dram_parameter("trimask", [128, 128], BF16,
                                        isOutput=False)
    eye = nc.declare_dram_parameter("eye", [64, 64], BF16, isOutput=False)
    out = nc.declare_dram_parameter("out", [S, D], BF16, isOutput=True)

    with tile.TileContext(nc) as tc, ExitStack() as ctx:
        sb = ctx.enter_context(tc.tile_pool(name="sb", bufs=1))
        wk_ = ctx.enter_context(tc.tile_pool(name="wk", bufs=2))
        pp = ctx.enter_context(tc.tile_pool(name="pp", bufs=1, space="PSUM"))

        # ---- persistent constants ----
        eye_sb = sb.tile([64, 64], BF16)
        nc.sync.dma_start(out=eye_sb, in_=eye[:, :])
        wq_sb = sb.tile([128, KB, QC], BF16)
        x_sb = sb.tile([128, KB, S], BF16)
        for kb in range(KB):
            nc.sync.dma_start(out=x_sb[:, kb, :], in_=xT[:, kb, :])
            nc.sync.dma_start(out=wq_sb[:, kb, :], in_=wq[:, kb, :])
        wkv_sb = sb.tile([128, KB, 128], BF16)
        nc.sync.dma_start(out=wkv_sb, in_=wkv[:, :, :])
        ctab_sb = sb.tile([128, S], BF16)
        nc.sync.dma_start(out=ctab_sb, in_=ctab[:, :])
        stab_sb = sb.tile([128, S], BF16)
        nc.sync.dma_start(out=stab_sb, in_=stab[:, :])
        wo_sb = sb.tile([128, 2, D], BF16)
        nc.sync.dma_start(out=wo_sb, in_=wo[:, :, :])
        tri_sb = sb.tile([128, 128], BF16)
        nc.sync.dma_start(out=tri_sb, in_=trimask[:, :])

        # PE warmup spin: keep the PE busy from t~1us so the HAM clock gate
        # opens (K=8/8) before the projection matmuls start, and bridge the
        # DMA-gated prologue.
        warm = pp.tile([64, 64], F32, name="warm", tag="po", bufs=1)
        for _ in range(64):
            nc.tensor.matmul(warm, lhsT=eye_sb, rhs=eye_sb,
                             start=True, stop=True)

        # ---- persistent activations ----
        qt0 = sb.tile([128, S], BF16)   # q^T heads 0,1 (roped)
        qt1 = sb.tile([128, S], BF16)   # q^T heads 2,3
        qts = [qt0, qt1]
        kt_sb = sb.tile([128, S], BF16)  # rows 0-63 k^T roped; 64-127 dup
        v_sb = sb.tile([128, KB, HD + 2], BF16)  # V natural + [ones, 0] cols
        ct0 = sb.tile([128, S], BF16)   # normalized ctx^T heads 0,1
        ct1 = sb.tile([128, S], BF16)
        cts = [ct0, ct1]
        nc.vector.memset(v_sb[:, :, HD:HD + 1], 1.0)
        nc.vector.memset(v_sb[:, :, HD + 1:HD + 2], 0.0)

        def emit_proj_q(c, u):
            """Q projection + rope for u-tile (2 heads) of chunk c."""
            cs = slice(c * SC, (c + 1) * SC)
            pq = pp.tile([128, SC], F32, name=f"pq_{c}_{u}", tag="ppq", bufs=1)
            for kb in range(KB):
                nc.tensor.matmul(
                    pq,
                    lhsT=wq_sb[:, kb, u * 128:(u + 1) * 128],
                    rhs=x_sb[:, kb, cs],
                    start=(kb == 0), stop=(kb == KB - 1),
                )
            qraw = wk_.tile([128, SC], BF16, name=f"qraw_{c}_{u}", tag="qraw",
                            bufs=2)
            nc.vector.tensor_copy(qraw, pq)
            qsw = wk_.tile([128, SC], BF16, name=f"qsw_{c}_{u}", tag="qsw",
                           bufs=2)
            nc.vector.stream_shuffle(qsw, qraw, SHUF)
            t1 = wk_.tile([128, SC], BF16, name=f"rt1_{c}_{u}", tag="rt1",
                          bufs=2)
            nc.vector.tensor_mul(t1, qraw, ctab_sb[:, cs])
            t2 = wk_.tile([128, SC], BF16, name=f"rt2_{c}_{u}", tag="rt2",
                          bufs=2)
            nc.vector.tensor_mul(t2, qsw, stab_sb[:, cs])
            nc.vector.tensor_add(qts[u][:, cs], t1, t2)

        def emit_proj_kv(c):
            """K/V projection for chunk c: rope K (+dup), V to natural."""
            cs = slice(c * SC, (c + 1) * SC)
            pkv = pp.tile([128, SC], F32, name=f"pkv_{c}", tag="ppq", bufs=1)
            for kb in range(KB):
                nc.tensor.matmul(
                    pkv,
                    lhsT=wkv_sb[:, kb, :],
                    rhs=x_sb[:, kb, cs],
                    start=(kb == 0), stop=(kb == KB - 1),
                )
            kvraw = wk_.tile([128, SC], BF16, name=f"kvraw_{c}", tag="qraw",
                             bufs=2)
            nc.vector.tensor_copy(kvraw, pkv)
            ksw = wk_.tile([64, SC], BF16, name=f"ksw_{c}", tag="ksw", bufs=2)
            nc.vector.stream_shuffle(ksw, kvraw[0:64, :], SHUF)
            k1 = wk_.tile([64, SC], BF16, name=f"kr1_{c}", tag="kr1", bufs=2)
            nc.vector.tensor_mul(k1, kvraw[0:64, :], ctab_sb[0:64, cs])
            k2 = wk_.tile([64, SC], BF16, name=f"kr2_{c}", tag="kr2", bufs=2)
            nc.vector.tensor_mul(k2, ksw, stab_sb[0:64, cs])
            nc.vector.tensor_add(kt_sb[0:64, cs], k1, k2)
            nc.sync.dma_start(out=kt_sb[64:128, cs], in_=kt_sb[0:64, cs])
            # V natural layout: move rows 64-127 down, PE-transpose per block
            vtr = wk_.tile([64, SC], BF16, name=f"vtr_{c}", tag="vtr", bufs=2)
            nc.sync.dma_start(out=vtr, in_=kvraw[64:128, :])
            for r in range(4):
                j = 4 * c + r
                pt = pp.tile([128, HD], BF16, name=f"pt_{c}_{r}", tag="sp",
                             bufs=2)
                nc.tensor.transpose(pt, vtr[:, r * 128:(r + 1) * 128], eye_sb)
                nc.vector.tensor_copy(v_sb[:, j, 0:HD], pt)

        def emit_outproj_half(c, mi, half):
            """Half (2 n-tiles) of one 128-query row block of the out proj."""
            m = 4 * c + mi
            mb = slice(m * 128, (m + 1) * 128)
            ob = wk_.tile([128, 2 * SC], BF16, name=f"ob_{c}_{mi}_{half}",
                          tag="ob", bufs=2)
            for ni in range(2):
                n = 2 * half + ni
                nck = slice(n * SC, (n + 1) * SC)
                po = pp.tile([128, SC], F32, name=f"po_{c}_{mi}_{n}", tag="po",
                             bufs=1)
                for u in range(2):
                    nc.tensor.matmul(
                        po,
                        lhsT=cts[u][:, mb],
                        rhs=wo_sb[:, u, nck],
                        start=(u == 0), stop=(u == 1),
                    )
                nc.vector.tensor_copy(ob[:, ni * SC:(ni + 1) * SC], po)
            nc.sync.dma_start(out=out[mb, half * 2 * SC:(half + 1) * 2 * SC],
                              in_=ob)

        def emit_attn(c, fillers):
            """Attention for chunk c; pops filler emitters to keep PE busy.
            A couple of fillers are held back to cover the normalize chain's
            latency at the end of the j-loop."""
            tail = [fillers.pop() for _ in range(min(2, len(fillers)))]
            njt = 4 * c + 4
            heads = [(u, idx) for u in (0, 1) for idx in (0, 1)]
            cps = {}
            for u, idx in heads:
                cps[(u, idx)] = pp.tile([HD + 2, SC], F32,
                                        name=f"cp_{c}_{u}_{idx}",
                                        tag=f"ctx{2 * u + idx}", bufs=1)
            es_for = {}

            def emit_scores(j):
                diag = j >= 4 * c
                r = j - 4 * c
                jb = slice(j * 128, (j + 1) * 128)
                lo = 128 * r if diag else 0
                nsl = slice(lo, SC)
                csl = slice(c * SC + lo, (c + 1) * SC)
                for u in (0, 1):
                    sps = []
                    for idx in (0, 1):
                        sp = pp.tile([128, SC], F32,
                                     name=f"sp_{c}_{u}_{j}_{idx}",
                                     tag="sp", bufs=2)
                        nc.tensor.matmul(
                            sp[:, nsl],
                            lhsT=kt_sb[idx * 64:idx * 64 + 64, jb],
                            rhs=qts[u][idx * 64:idx * 64 + 64, csl],
                            start=True, stop=True,
                            tile_position=(idx * 64, 0),
                        )
                        sps.append(sp)
                    for idx in (0, 1):
                        e = wk_.tile([128, SC], BF16,
                                     name=f"e_{c}_{u}_{j}_{idx}",
                                     tag="es", bufs=8)
                        nc.scalar.activation(e[:, nsl], sps[idx][:, nsl],
                                             AF.Exp, scale=SCALE)
                        if diag:
                            dsl = slice(lo, lo + 128)
                            nc.vector.tensor_mul(e[:, dsl], e[:, dsl], tri_sb)
                        es_for[(u, idx, j)] = (e, nsl)

            def emit_ctx(j):
                for u, idx in heads:
                    e, nsl = es_for.pop((u, idx, j))
                    nc.tensor.matmul(
                        cps[(u, idx)][:, nsl],
                        lhsT=v_sb[:, j, :],
                        rhs=e[:, nsl],
                        start=(j == 0), stop=(j == njt - 1),
                    )

            emit_scores(0)
            for j in range(njt):
                if j + 1 < njt:
                    emit_scores(j + 1)
                emit_ctx(j)
                if fillers:
                    fillers.pop(0)()
            # normalize: cts = ctx / den via recip-broadcast-multiply
            cs = slice(c * SC, (c + 1) * SC)
            for u, idx in heads:
                cp = cps[(u, idx)]
                # NOTE: gpsimd.partition_broadcast must read partition 0 on
                # real HW (reading a sliced row at partition 64 simulates
                # fine but returns garbage on silicon), so the denominator
                # row is first moved to partition 0 with a small DMA.
                scr = wk_.tile([HD + 1, SC], F32,
                               name=f"scr_{c}_{u}_{idx}", tag="scr", bufs=4)
                nc.scalar.copy(scr[HD:HD + 1, :], cp[HD:HD + 1, :])
                den0 = wk_.tile([1, SC], F32, name=f"den_{c}_{u}_{idx}",
                                tag="den", bufs=4)
                nc.sync.dma_start(out=den0, in_=scr[HD:HD + 1, :])
                rec0 = wk_.tile([1, SC], F32, name=f"rec_{c}_{u}_{idx}",
                                tag="rec", bufs=4)
                nc.vector.reciprocal_approx_fast(out=rec0, in_=den0)
                bcf = wk_.tile([64, SC], F32, name=f"bcf_{c}_{u}_{idx}",
                               tag="bcf", bufs=4)
                nc.gpsimd.partition_broadcast(bcf, rec0[0:1, :])
                rsl = slice(idx * 64, idx * 64 + 64)
                nc.vector.scalar_tensor_tensor(
                    cts[u][rsl, cs], cp[0:HD, :], 1.0, bcf,
                    mybir.AluOpType.mult, mybir.AluOpType.mult,
                )
            for f in tail:
                f()
            while fillers:
                fillers.pop(0)()

        # ---- schedule ----
        # proj(0)+proj(1) upfront: dense PE work that warms the HAM while x
        # streams in; proj(c+2) + outproj(c-1) interleave into attn(c)'s
        # j-loop, weighted toward the later (longer, exp-bound) chunks.
        for cc in (0, 1):
            emit_proj_q(cc, 0)
            emit_proj_q(cc, 1)
            emit_proj_kv(cc)
        for c in range(NSC):
            fillers = []
            if c + 2 < NSC:
                fillers.append(lambda cc=c + 2: emit_proj_q(cc, 0))
                fillers.append(lambda cc=c + 2: emit_proj_q(cc, 1))
                fillers.append(lambda cc=c + 2: emit_proj_kv(cc))
            if c > 0:
                for mi in range(4):
                    for half in (0, 1):
                        fillers.append(
                            lambda cc=c - 1, m=mi, h=half:
                            emit_outproj_half(cc, m, h))
            # alternate proj/outproj units so neither starves
            if len(fillers) > 8:
                proj_f, op_f = fillers[:3], fillers[3:]
                mixed = []
                while proj_f or op_f:
                    if op_f:
                        mixed.append(op_f.pop(0))
                    if op_f:
                        mixed.append(op_f.pop(0))
                    if proj_f:
                        mixed.append(proj_f.pop(0))
                fillers = mixed
            emit_attn(c, fillers)
        for mi in range(4):
            for half in (0, 1):
                emit_outproj_half(NSC - 1, mi, half)

    nc.finalize()
    return nc


def _get_nc():
    global _NC
    if _NC is None:
        _NC = _build()
    return _NC


def _rope_perm():
    """Head-local (64) permutation: pair (x1_i, x2_i) -> 16 apart in a
    32-partition quadrant. newpos[old] for old in 0..63."""
    newpos = np.empty(64, dtype=np.int64)
    for i in range(32):
        newpos[i] = (i // 16) * 32 + (i % 16)           # x1_i
        newpos[32 + i] = (i // 16) * 32 + 16 + (i % 16)  # x2_i
    return newpos


def _prep_in_maps(x, Wq, Wk, Wv, Wo, cos, sin):
    import ml_dtypes
    bf = ml_dtypes.bfloat16
    x0 = np.asarray(x, np.float32).reshape(S, D)
    xT = np.ascontiguousarray(
        x0.T.reshape(KB, 128, S).transpose(1, 0, 2)).astype(bf)

    newpos = _rope_perm()
    # permutation as gather: perm_src[new] = old
    perm_src = np.empty(64, dtype=np.int64)
    perm_src[newpos] = np.arange(64)

    # rope tables in the permuted layout (pattern has period 64)
    cosT = np.asarray(cos, np.float32).T  # (32, S)
    sinT = np.asarray(sin, np.float32).T
    ctab64 = np.empty((64, S), np.float32)
    stab64 = np.empty((64, S), np.float32)
    for p in range(64):
        quad, off = p // 32, p % 32
        i = quad * 16 + (off % 16)
        is_x2 = off >= 16
        ctab64[p] = cosT[i]
        stab64[p] = sinT[i] if is_x2 else -sinT[i]
    ctab = np.tile(ctab64, (2, 1)).astype(bf)
    stab = np.tile(stab64, (2, 1)).astype(bf)

    trimask = (np.arange(128)[:, None] <= np.arange(128)[None, :]).astype(bf)
    eye = np.eye(64, dtype=np.float32).astype(bf)

    Wq = np.asarray(Wq, np.float32)
    Wk = np.asarray(Wk, np.float32)
    Wv = np.asarray(Wv, np.float32)
    Wo = np.asarray(Wo, np.float32)
    # apply rope perm within each head's 64 columns
    Wq_p = Wq.reshape(D, 32, 64)[:, :, perm_src].reshape(D, D)
    Wk_p = Wk.reshape(D, 8, 64)[:, :, perm_src].reshape(D, 8 * 64)

    in_maps = []
    for i in range(NCORES):
        wq_i = np.ascontiguousarray(
            Wq_p[:, i * QC:(i + 1) * QC].reshape(KB, 128, QC)
            .transpose(1, 0, 2)).astype(bf)
        wkv_i = np.concatenate(
            [Wk_p[:, i * HD:(i + 1) * HD], Wv[:, i * HD:(i + 1) * HD]],
            axis=1)
        wkv_i = np.ascontiguousarray(
            wkv_i.reshape(KB, 128, 128).transpose(1, 0, 2)).astype(bf)
        wo_i = np.ascontiguousarray(
            Wo[i * QC:(i + 1) * QC, :].reshape(2, 128, D)
            .transpose(1, 0, 2)).astype(bf)
        in_maps.append({
            "xT": xT, "wq": wq_i, "wkv": wkv_i, "wo": wo_i,
            "ctab": ctab, "stab": stab, "trimask": trimask, "eye": eye,
        })
    return in_maps


def run(inputs, **kw):
    nc = _get_nc()
    in_maps = _prep_in_maps(**inputs)
    return run_bass_kernel_spmd(nc, in_maps, list(range(NCORES)), **kw)


def kernel(x, Wq, Wk, Wv, Wo, cos, sin):
    res = run(dict(x=x, Wq=Wq, Wk=Wk, Wv=Wv, Wo=Wo, cos=cos, sin=sin))
    acc = np.zeros((S, D), np.float32)
    for r in res.results:
        acc += r["out"].astype(np.float32)
    return acc.reshape(1, S, D)
